# revision 20
# baseline (speedup 1.0000x reference)
"""AutoDOAS forward model on 8 TRN2 NeuronCores — conv-first data-parallel.

Key restructure vs the naive pipeline:
  1. The per-row 15-tap Gaussian LSF conv is folded into the differential
     matmul: conv commutes (to ~1e-3) with the per-row affine resample, and
     the Gaussian kernel family over sigma in [0.2, 5] is rank-~6.  The host
     pre-convolves the 18-row basis matrix M with 6 SVD basis kernels; the
     device projects each row's exact kernel onto the basis (tiny matmul) and
     builds a [109, 128] stationary so ONE bf16 matmul produces the
     smoothed+centered signal C directly.
  2. The per-row warp uses per-(row,block) integer window gathers (batched:
     32 windows per indirect DMA instruction, so the 994ns SWDGE fixed cost
     is paid 4x per core instead of 128x) and an exact 3-point
     min/relu lerp: out = win2 + relu(rho)*t2s + min(rho,0)*t1s.
  3. Epilogue folded to exp + one fused multiply-accumulate pass + one
     per-row-affine pass (gain into exp bias; nonlinearity+stray via
     quadratic-in-cnt coefficients; stray mean from the AMR accumulators).
"""
import sys

sys.path.insert(0, "/opt/trn_rl_repo")

import numpy as np
import ml_dtypes
from concourse import bass, bacc, mybir, tile
from concourse.bass_utils import run_bass_kernel_spmd
from concourse.masks import make_identity

B, W, G, E, NI, K = 2048, 8192, 8, 128, 16, 15
NCORES = 8
BS = B // NCORES            # 256 rows per core
NT, TP = 2, 128             # tiles per core, rows per tile
M = 6                       # sigma-basis rank
CROWS = 18 * M              # matmul contract rows
BLK = 256                   # output columns per window
NBLK = W // BLK             # 32 windows per row
LWIN = 262                  # gathered elems per window
WPIT = 264                  # window pitch in SBUF
PADL, PADR = 176, 180
ROWW = PADL + W + PADR      # 8548 padded row width
AW = 4480                   # scratch half A: cols [0, AW)
BOFF = 3904                 # scratch half B: cols [BOFF, ROWW)
BW = ROWW - BOFF            # 4644
CHB = 8                     # windows per combine chunk
CHW = CHB * BLK             # 2048
NCH = NBLK // CHB           # 4 combine chunks per tile

F32 = mybir.dt.float32
BF16 = mybir.dt.bfloat16
I32 = mybir.dt.int32
AF = mybir.ActivationFunctionType
OP = mybir.AluOpType

_CACHE = {}


def _bcast(ap, reps):
    return ap.to_broadcast([ap.shape[0], reps])


def _gkern(sig, nk=K):
    half = (nk - 1) / 2
    pos = np.linspace(-half, half, nk)
    k = np.exp(-0.5 * (pos[None, :] / (sig[:, None] + 1e-6)) ** 2)
    return k / k.sum(1, keepdims=True)


def _basis():
    sg = np.exp(np.linspace(np.log(0.2), np.log(5.0), 4000))
    _, _, Vt = np.linalg.svd(_gkern(sg), full_matrices=False)
    return np.ascontiguousarray(Vt[:M].T)          # [15, M]


def _build(wl0, dlam):
    nc = bacc.Bacc("TRN2", target_bir_lowering=False, debug=False, num_devices=8)

    uT_e = nc.declare_dram_parameter("uT", [18, BS], F32, isOutput=False)
    nuisT_e = nc.declare_dram_parameter("nuisT", [G, BS], F32, isOutput=False)
    oh_e = nc.declare_dram_parameter("onehotT", [NI, BS], F32, isOutput=False)
    emb_e = nc.declare_dram_parameter("embT16", [NI, E], F32, isOutput=False)
    w1a_e = nc.declare_dram_parameter("w1a", [E, 64], F32, isOutput=False)
    w1b_e = nc.declare_dram_parameter("w1b", [G, 64], F32, isOutput=False)
    b1r_e = nc.declare_dram_parameter("b1r", [1, 64], F32, isOutput=False)
    w2s_e = nc.declare_dram_parameter("w2s", [65, 7], F32, isOutput=False)
    Mst_e = nc.declare_dram_parameter("MstackP", [CROWS, ROWW], BF16,
                                      isOutput=False)
    nMb_e = nc.declare_dram_parameter("negMbar", [CROWS, 1], BF16,
                                      isOutput=False)
    Qs_e = nc.declare_dram_parameter("Qs", [K, M], F32, isOutput=False)
    qsum_e = nc.declare_dram_parameter("qsum", [M, 1], F32, isOutput=False)
    selS_e = nc.declare_dram_parameter("selS", [M, CROWS], F32,
                                       isOutput=False)
    selG_e = nc.declare_dram_parameter("selG", [18, CROWS], F32,
                                       isOutput=False)
    out_e = nc.declare_dram_parameter("out", [BS, W], BF16, isOutput=True)

    SCB = [0, 2048, 3904, 6080]          # scratch quarter col bases
    SCW = [2432, 2432, 2592, 2468]       # scratch quarter widths
    scr = [[nc.dram_tensor(f"scr{t}q{q}", [TP, SCW[q]], BF16)
            for q in range(4)] for t in range(NT)]

    with tile.TileContext(nc) as tc:
        with (
            tc.tile_pool(name="const", bufs=1) as cp,
            tc.tile_pool(name="small", bufs=2) as sp,
            tc.tile_pool(name="prm", bufs=2) as pp,
            tc.tile_pool(name="win", bufs=2) as wp,
            tc.tile_pool(name="t1w", bufs=2) as tp1,
            tc.tile_pool(name="cmb", bufs=2) as cb,
            tc.tile_pool(name="vq", bufs=1) as vqp,
            tc.tile_pool(name="fin", bufs=2) as fp,
            tc.tile_pool(name="bnc", bufs=1) as bncp,
            tc.tile_pool(name="psMM", bufs=2, space="PSUM") as psMM,
            tc.tile_pool(name="psU", bufs=2, space="PSUM") as psU,
            tc.tile_pool(name="psC", bufs=2, space="PSUM") as psC,
        ):
            # ---------- constants / inputs ----------
            uTf = cp.tile([18, BS], F32)
            nc.sync.dma_start(uTf[:], uT_e[:])
            nuisT = cp.tile([G, BS], F32)
            nc.sync.dma_start(nuisT[:], nuisT_e[:])
            oh = cp.tile([NI, BS], F32)
            nc.sync.dma_start(oh[:], oh_e[:])
            emb16 = cp.tile([NI, E], F32)
            nc.sync.dma_start(emb16[:], emb_e[:])
            w1a = cp.tile([E, 64], F32)
            nc.sync.dma_start(w1a[:], w1a_e[:])
            w1b = cp.tile([G, 64], F32)
            nc.sync.dma_start(w1b[:], w1b_e[:])
            b1r = cp.tile([1, 64], F32)
            nc.sync.dma_start(b1r[:], b1r_e[:])
            w2s = cp.tile([65, 7], F32)
            nc.sync.dma_start(w2s[:], w2s_e[:])
            Mst = cp.tile([CROWS, ROWW], BF16)
            nc.gpsimd.dma_start(Mst[:], Mst_e[:])
            nMb = cp.tile([CROWS, 1], BF16)
            nc.sync.dma_start(nMb[:], nMb_e[:])
            Qs = cp.tile([K, M], F32)
            nc.sync.dma_start(Qs[:], Qs_e[:])
            qsum = cp.tile([M, 1], F32)
            nc.sync.dma_start(qsum[:], qsum_e[:])
            selS = cp.tile([M, CROWS], F32)
            nc.sync.dma_start(selS[:], selS_e[:])
            selG = cp.tile([18, CROWS], F32)
            nc.sync.dma_start(selG[:], selG_e[:])

            identF = cp.tile([TP, TP], F32)
            make_identity(nc, identF[:])
            identB = cp.tile([TP, TP], BF16)
            make_identity(nc, identB[:])
            ones18 = cp.tile([1, 18], F32)
            nc.vector.memset(ones18[:], 1.0)
            onesB = cp.tile([1, BS], F32)
            nc.vector.memset(onesB[:], 1.0)
            negone = cp.tile([TP, 1], F32)
            nc.vector.memset(negone[:], -1.0)

            xmod256 = cp.tile([TP, BLK], BF16)  # 0..255 (one block)
            nc.gpsimd.iota(xmod256[:], [[1, BLK]], channel_multiplier=0,
                           allow_small_or_imprecise_dtypes=True)
            blkio = cp.tile([TP, NBLK], F32)   # 256*b
            nc.gpsimd.iota(blkio[:], [[BLK, NBLK]], channel_multiplier=0,
                           allow_small_or_imprecise_dtypes=True)
            rowq = []
            for q in range(4):
                rq = cp.tile([TP, CHB], I32)   # p*W_q - 1 - colbase_q
                nc.gpsimd.iota(rq[:], [[0, CHB]], base=-1 - SCB[q],
                               channel_multiplier=SCW[q])
                rowq.append(rq)
            kio = cp.tile([TP, K], F32)        # -7..7
            nc.gpsimd.iota(kio[:], [[1, K]], base=-7, channel_multiplier=0,
                           allow_small_or_imprecise_dtypes=True)

            # embT [128, BS] = emb16.T @ onehotT
            embps = psU.tile([TP, 512], F32, tag="u")
            nc.tensor.matmul(embps[0:E, 0:BS], emb16[:], oh[:], start=True,
                             stop=True)
            embT = cp.tile([E, BS], F32)
            nc.scalar.activation(embT[:], embps[0:E, 0:BS], AF.Copy)

            # ---------- phase 1 params ----------
            # Both tiles' Gelu back-to-back (only set-10 users); everything
            # else (Tanh/Exp/Square/Identity/Copy) lives in act set 0.
            P = {t: {} for t in range(NT)}
            for t in range(NT):
                rs = t * TP
                hps = psU.tile([TP, 512], F32, tag="u")
                nc.tensor.matmul(hps[:, 0:64], embT[:, rs:rs + TP], w1a[:],
                                 start=True, stop=False)
                nc.tensor.matmul(hps[:, 0:64], nuisT[:, rs:rs + TP], w1b[:],
                                 start=False, stop=False)
                nc.tensor.matmul(hps[:, 0:64], onesB[:, rs:rs + TP], b1r[:],
                                 start=False, stop=True)
                h = sp.tile([TP, 64], F32, tag=f"h{t}")
                nc.scalar.activation(h[:], hps[:, 0:64], AF.Gelu)
                P[t]["h"] = h
            for t in range(NT):
                rs = t * TP
                hTp = psU.tile([TP, 512], F32, tag="u")
                nc.tensor.transpose(hTp[0:64, 0:TP], P[t]["h"][:], identF[:])
                hT1 = sp.tile([65, TP], F32, tag="hT1")
                nc.scalar.activation(hT1[0:64, :], hTp[0:64, 0:TP], AF.Copy)
                nc.vector.memset(hT1[64:65, :], 1.0)
                pps = psU.tile([TP, 512], F32, tag="u")
                nc.tensor.matmul(pps[:, 0:7], hT1[:], w2s[:], start=True,
                                 stop=True)
                pv = pp.tile([TP, 7], F32, tag="pv")
                nc.scalar.activation(pv[:], pps[:, 0:7], AF.Copy)

                th23 = sp.tile([TP, 2], F32, tag="th23")
                nc.scalar.activation(th23[:], pv[:, 2:4], AF.Tanh)
                th2, th3 = th23[:, 0:1], th23[:, 1:2]
                st_s = pp.tile([TP, 1], F32, tag="st_s")
                nc.scalar.activation(st_s[:], pv[:, 5:6], AF.Tanh, scale=0.5)
                nc.vector.tensor_scalar(st_s[:], st_s[:], 0.025, 0.025,
                                        OP.mult, OP.add)
                nonl = pp.tile([TP, 1], F32, tag="nonl")
                nc.scalar.activation(nonl[:], pv[:, 6:7], AF.Tanh)
                nc.vector.tensor_scalar(nonl[:], nonl[:], 0.02, None, OP.mult)
                a_sl = pp.tile([TP, 1], F32, tag="a_sl")
                nc.vector.tensor_scalar(a_sl[:], th3, 0.005, None, OP.mult)
                ws_s = pp.tile([TP, 1], F32, tag="ws_s")
                nc.vector.tensor_scalar(ws_s[:], th3, 0.005, 1.0,
                                        OP.mult, OP.add)
                tsh = pp.tile([TP, 1], F32, tag="tsh")
                nc.vector.tensor_scalar(tsh[:], th3, 0.005 * wl0 / dlam,
                                        None, OP.mult)
                nc.vector.scalar_tensor_tensor(tsh[:], th2, 0.05 / dlam,
                                               tsh[:], OP.mult, OP.add)
                omst = pp.tile([TP, 1], F32, tag="omst")
                nc.vector.tensor_scalar(omst[:], st_s[:], -1.0, 1.0,
                                        OP.mult, OP.add)
                orec = sp.tile([TP, 1], F32, tag="orec")
                nc.vector.reciprocal(orec[:], omst[:])
                ratio = pp.tile([TP, 1], F32, tag="ratio")
                nc.vector.tensor_tensor(ratio[:], st_s[:], orec[:], OP.mult)

                # window bases + fractional phase
                pst = sp.tile([TP, NBLK], F32, tag="pst")
                nc.vector.scalar_tensor_tensor(pst[:], blkio[:], ws_s[:, 0:1],
                                               _bcast(tsh[:, 0:1], NBLK),
                                               OP.mult, OP.add)
                am1 = sp.tile([TP, 1], F32, tag="am1")
                nc.vector.tensor_scalar(am1[:], a_sl[:], float(BLK - 1), 0.0,
                                        OP.mult, OP.min)
                nc.vector.tensor_scalar(am1[:], am1[:], 175.5, None, OP.add)
                pmf = sp.tile([TP, NBLK], F32, tag="pmf")
                nc.vector.tensor_scalar(pmf[:], pst[:], am1[:, 0:1], None,
                                        OP.add)
                bi = sp.tile([TP, NBLK], I32, tag="bi")
                nc.vector.tensor_copy(bi[:], pmf[:])
                bif = sp.tile([TP, NBLK], F32, tag="bif")
                nc.vector.tensor_copy(bif[:], bi[:])
                phif = sp.tile([TP, NBLK], F32, tag="phif")
                nc.vector.tensor_tensor(phif[:], pst[:], bif[:], OP.subtract)
                phib = pp.tile([TP, NBLK], F32, tag="phib")
                nc.vector.tensor_scalar(phib[:], phif[:], 175.0, None, OP.add)
                phim1 = pp.tile([TP, NBLK], F32, tag="phim1")
                nc.vector.tensor_scalar(phim1[:], phib[:], -1.0, None, OP.add)
                idx = pp.tile([TP, NBLK], I32, tag="idx")
                for q in range(4):
                    nc.vector.tensor_tensor(idx[:, CHB * q:CHB * (q + 1)],
                                            rowq[q][:],
                                            bi[:, CHB * q:CHB * (q + 1)],
                                            OP.add)

                d = P[t]
                d.update(pv=pv, st_s=st_s, nonl=nonl, a_sl=a_sl, omst=omst,
                         ratio=ratio, phib=phib, phim1=phim1, idx=idx)

            # ---------- phase 2 params ----------
            # softplus(x) = relu(x) + poly(exp(-|x|)), poly ~ ln(1+t) on [0,1]
            SC = [0.0009875142, 0.9914986, -0.4436204, 0.1572681, -0.0187619]

            # one merged softplus over [lsf0, gain0, lsf1, gain1]
            spin = sp.tile([TP, 2 * NT], F32, tag="spin")
            for t in range(NT):
                nc.vector.tensor_copy(spin[:, 2 * t:2 * t + 1],
                                      P[t]["pv"][:, 4:5])
                nc.vector.tensor_copy(spin[:, 2 * t + 1:2 * t + 2],
                                      P[t]["pv"][:, 0:1])
            spu = sp.tile([TP, 2 * NT], F32, tag="spu")
            nc.vector.tensor_scalar(spu[:], spin[:], -1.0, None, OP.mult)
            nc.vector.tensor_tensor(spu[:], spu[:], spin[:], OP.max)
            spt = sp.tile([TP, 2 * NT], F32, tag="spt")
            nc.scalar.activation(spt[:], spu[:], AF.Exp, scale=-1.0)
            spo = sp.tile([TP, 2 * NT], F32, tag="spo")
            nc.vector.tensor_scalar(spo[:], spt[:], SC[4], SC[3],
                                    OP.mult, OP.add)
            nc.vector.scalar_tensor_tensor(spo[:], spo[:], 1.0, spt[:],
                                           OP.mult, OP.mult)
            nc.vector.tensor_scalar(spo[:], spo[:], SC[2], None, OP.add)
            nc.vector.scalar_tensor_tensor(spo[:], spo[:], 1.0, spt[:],
                                           OP.mult, OP.mult)
            nc.vector.tensor_scalar(spo[:], spo[:], SC[1], None, OP.add)
            nc.vector.scalar_tensor_tensor(spo[:], spo[:], 1.0, spt[:],
                                           OP.mult, OP.mult)
            nc.vector.tensor_scalar(spo[:], spo[:], SC[0], None, OP.add)
            spr = sp.tile([TP, 2 * NT], F32, tag="spr")
            nc.vector.tensor_scalar(spr[:], spin[:], 0.0, None, OP.max)
            nc.vector.tensor_tensor(spr[:], spr[:], spo[:], OP.add)

            for t in range(NT):
                rs = t * TP
                d = P[t]
                pv = d["pv"]

                lsf = sp.tile([TP, 1], F32, tag="lsf")
                nc.vector.tensor_scalar(lsf[:], spr[:, 2 * t:2 * t + 1],
                                        0.001, 5.0, OP.add, OP.min)
                nc.vector.tensor_scalar(lsf[:], lsf[:], 0.2, 1e-6,
                                        OP.max, OP.add)
                linv = sp.tile([TP, 1], F32, tag="linv")
                nc.vector.reciprocal(linv[:], lsf[:])
                kern = sp.tile([TP, K], F32, tag="kern")
                nc.vector.tensor_scalar(kern[:], kio[:], linv[:, 0:1], None,
                                        OP.mult)
                nc.scalar.activation(kern[:], kern[:], AF.Square)
                nc.scalar.activation(kern[:], kern[:], AF.Exp, scale=-0.5)
                ksum = sp.tile([TP, 1], F32, tag="ksum")
                nc.vector.tensor_reduce(ksum[:], kern[:], mybir.AxisListType.X,
                                        OP.add)
                krec = sp.tile([TP, 1], F32, tag="krec")
                nc.vector.reciprocal(krec[:], ksum[:])
                nc.vector.tensor_scalar(kern[:], kern[:], krec[:, 0:1], None,
                                        OP.mult)

                # project normalized kernel onto basis -> cT [M, TP]
                kTp = psU.tile([TP, 512], F32, tag="u")
                nc.tensor.transpose(kTp[0:K, 0:TP], kern[:], identF[:])
                kTs = sp.tile([K, TP], F32, tag="kTs")
                nc.scalar.activation(kTs[:], kTp[0:K, 0:TP], AF.Copy)
                cps = psU.tile([TP, 512], F32, tag="u")
                nc.tensor.matmul(cps[0:M, 0:TP], Qs[:], kTs[:], start=True,
                                 stop=True)
                cTs = sp.tile([M, TP], F32, tag="cTs")
                nc.scalar.activation(cTs[:], cps[0:M, 0:TP], AF.Copy)
                sps = psU.tile([TP, 512], F32, tag="u")
                nc.tensor.matmul(sps[0:1, 0:TP], qsum[:], cTs[:], start=True,
                                 stop=True)
                ssb = sp.tile([1, TP], F32, tag="ssb")
                nc.scalar.activation(ssb[:], sps[0:1, 0:TP], AF.Copy)
                rs_ = sp.tile([1, TP], F32, tag="rs_")
                nc.vector.reciprocal(rs_[:], ssb[:])
                rsB = psU.tile([TP, 512], F32, tag="u")
                nc.tensor.matmul(rsB[0:18, 0:TP], ones18[:], rs_[:],
                                 start=True, stop=True)
                uTs = sp.tile([18, TP], F32, tag="uTs")
                nc.vector.tensor_tensor(uTs[:], uTf[:, rs:rs + TP],
                                        rsB[0:18, 0:TP], OP.mult)
                vT = pp.tile([CROWS, TP], BF16, tag="vT")
                cBall = psU.tile([TP, 512], F32, tag="u")
                nc.tensor.matmul(cBall[0:CROWS, 0:TP], selS[:], cTs[:],
                                 start=True, stop=True)
                uAll = psU.tile([TP, 512], F32, tag="u")
                nc.tensor.matmul(uAll[0:CROWS, 0:TP], selG[:], uTs[:],
                                 start=True, stop=True)
                uAllS = sp.tile([CROWS, TP], F32, tag="uAllS")
                nc.scalar.activation(uAllS[:], uAll[0:CROWS, 0:TP],
                                     AF.Copy)
                nc.vector.tensor_tensor(vT[:], cBall[0:CROWS, 0:TP],
                                        uAllS[:], OP.mult)
                # negC0 per row, column layout: vT.T @ negMbar
                nps = psU.tile([TP, 512], F32, tag="u")
                nc.tensor.matmul(nps[:, 0:1], vT[:], nMb[:],
                                 start=True, stop=True)
                nC0 = pp.tile([TP, 1], F32, tag="nC0")
                nc.vector.tensor_copy(nC0[:], nps[:, 0:1])
                gain = sp.tile([TP, 1], F32, tag="gain")
                nc.vector.tensor_scalar(gain[:], spr[:, 2 * t + 1:2 * t + 2],
                                        0.001, None, OP.add)

                # per-tile slope ramp a_sl*x over one block (bf16, 4x TS)
                slramp = pp.tile([TP, BLK], BF16, tag="slramp")
                nc.vector.tensor_scalar(slramp[:], xmod256[:],
                                        d["a_sl"][:, 0:1], None, OP.mult)
                d["slramp"] = slramp

                # epilogue quadratic coefficients (gain folded in)
                off = pv[:, 1:2]
                q1 = sp.tile([TP, 1], F32, tag="q1")
                nc.vector.tensor_tensor(q1[:], off, d["nonl"][:], OP.mult)
                q2 = sp.tile([TP, 1], F32, tag="q2")
                nc.vector.tensor_scalar(q2[:], q1[:], 2.0, 1.0, OP.mult, OP.add)
                Bt = pp.tile([TP, 1], F32, tag="Bt")
                nc.vector.tensor_tensor(Bt[:], d["omst"][:], q2[:], OP.mult)
                nc.vector.tensor_tensor(Bt[:], Bt[:], gain[:], OP.mult)
                At = pp.tile([TP, 1], F32, tag="At")
                nc.vector.tensor_tensor(At[:], d["omst"][:], d["nonl"][:],
                                        OP.mult)
                nc.vector.tensor_tensor(At[:], At[:], gain[:], OP.mult)
                nc.vector.tensor_tensor(At[:], At[:], gain[:], OP.mult)
                q3 = sp.tile([TP, 1], F32, tag="q3")
                nc.vector.tensor_tensor(q3[:], off, q1[:], OP.mult)
                nc.vector.tensor_tensor(q3[:], q3[:], off, OP.add)
                Ct = pp.tile([TP, 1], F32, tag="Ct")
                nc.vector.tensor_tensor(Ct[:], d["omst"][:], q3[:], OP.mult)
                # clamp |At| >= 3e-5 (keep sign), q = Bt/(2*Atc), aq = Atc*q^2
                sg = sp.tile([TP, 1], F32, tag="sg")
                nc.vector.tensor_scalar(sg[:], At[:], 0.0, None, OP.is_ge)
                nc.vector.tensor_scalar(sg[:], sg[:], 2.0, -1.0, OP.mult,
                                        OP.add)
                Atc = pp.tile([TP, 1], F32, tag="Atc")
                nc.vector.tensor_tensor(Atc[:], At[:], sg[:], OP.mult)
                nc.vector.tensor_scalar(Atc[:], Atc[:], 3e-5, None, OP.max)
                nc.vector.tensor_tensor(Atc[:], Atc[:], sg[:], OP.mult)
                arec = sp.tile([TP, 1], F32, tag="arec")
                nc.vector.reciprocal(arec[:], Atc[:])
                qb = pp.tile([TP, 1], F32, tag="qb")
                nc.vector.tensor_tensor(qb[:], Bt[:], arec[:], OP.mult)
                nc.vector.tensor_scalar(qb[:], qb[:], 0.5, None, OP.mult)
                aq = pp.tile([TP, 1], F32, tag="aq")
                nc.vector.tensor_tensor(aq[:], qb[:], qb[:], OP.mult)
                nc.vector.tensor_tensor(aq[:], aq[:], Atc[:], OP.mult)
                d.update(vT=vT, b1e=nC0, At=At, Bt=Bt, Ct=Ct, nC0=nC0,
                         Atc=Atc, qb=qb, aq=aq)

            # ---------- main loop ----------
            MMW = 1024
            bounds = list(range(0, ROWW, MMW)) + [ROWW]

            # stage A: both tiles' big matmul -> SBUF assembly -> scratch DMAs
            for t in range(NT):
                d = P[t]
                vT, nC0 = d["vT"], d["nC0"]
                d4q = []
                for q in range(4):
                    d4qt = bncp.tile([TP, SCW[q]], BF16, tag=f"d4q{q}")
                    d4q.append(d4qt)
                for k in range(len(bounds) - 1):
                    c0, c1 = bounds[k], bounds[k + 1]
                    w = c1 - c0
                    dps = psMM.tile([TP, MMW], F32, tag="mm")
                    pos = 0
                    while pos < w:
                        n = min(512, w - pos)
                        nc.tensor.matmul(dps[:, pos:pos + n], vT[:],
                                         Mst[:, c0 + pos:c0 + pos + n],
                                         start=True, stop=True)
                        pos += n
                    for q in range(4):
                        o0, o1 = max(c0, SCB[q]), min(c1, SCB[q] + SCW[q])
                        if o0 < o1:
                            nc.scalar.activation(
                                d4q[q][:, o0 - SCB[q]:o1 - SCB[q]],
                                dps[:, o0 - c0:o1 - c0],
                                AF.Identity, bias=nC0[:, 0:1])
                for q in range(4):
                    nc.sync.dma_start(scr[t][q][:], d4q[q][:])

            # stage B: tile-interleaved chunk pipeline. Gathers for (t, c+1)
            # are issued before combine (t, c); combines alternate t0/t1 so
            # each tile's DVE work overlaps the other tile's Pool gathers.
            vqs, msums = [], []
            for t in range(NT):
                vq_t = vqp.tile([TP, W], F32, tag=f"vq{t}")
                msum_t = sp.tile([TP, NCH], F32, tag=f"msum{t}")
                vqs.append(vq_t)
                msums.append(msum_t)

            def gather_chunk(t, c):
                # NOTE: one offset column per instruction — the HW lowering
                # only honors a single offset per partition (multi-column
                # offset APs gather garbage).
                idx = P[t]["idx"]
                wt = wp.tile([TP, CHB * WPIT], BF16, tag="win")
                for r in range(CHB):
                    nc.gpsimd.indirect_dma_start(
                        out=wt[:, WPIT * r:WPIT * r + LWIN],
                        out_offset=None,
                        in_=scr[t][c][:],
                        in_offset=bass.IndirectOffsetOnAxis(
                            ap=idx[:, CHB * c + r:CHB * c + r + 1],
                            axis=1),
                    )
                return wt

            def combine(t, c, wt):
                d = P[t]
                phib, phim1, slramp = d["phib"], d["phim1"], d["slramp"]
                b1e, qb = d["b1e"], d["qb"]
                t1w = tp1.tile([TP, CHB * WPIT], BF16, tag="t1w")
                nc.vector.tensor_tensor(t1w[:, 0:CHB * WPIT - 1],
                                        wt[:, 1:], wt[:, :-1],
                                        OP.subtract)
                d2w = tp1.tile([TP, CHB * WPIT], BF16, tag="d2w")
                nc.vector.tensor_tensor(d2w[:, 0:CHB * WPIT - 2],
                                        t1w[:, 1:CHB * WPIT - 1],
                                        t1w[:, 0:CHB * WPIT - 2],
                                        OP.subtract)

                def wsl(tl, j):
                    return tl[:].rearrange("p (b l) -> p b l", l=WPIT)[
                        :, :, j:j + BLK]

                # out = win2 + min(rho,0)*t1s + relu(rho)*t2s
                #       + relu(rho-1)*d2w[x+2],  rho = a_sl*x + phi_b
                # (relu(rho-2) tap is identically zero: rho < 1.775 for
                #  |a_sl| <= 0.005, BLK=256)
                # coeffs built per 256-block from the shared slope ramp
                # (per-partition scalar phi) — all TS ops at 4x rate.
                mn = cb.tile([TP, CHW], BF16, tag="mn")
                r0 = cb.tile([TP, CHW], BF16, tag="r0")
                r1 = cb.tile([TP, CHW], BF16, tag="r1")
                for b in range(CHB):
                    col = CHB * c + b
                    sl_ = slice(BLK * b, BLK * (b + 1))
                    nc.vector.tensor_scalar(mn[:, sl_], slramp[:],
                                            phib[:, col:col + 1], 0.0,
                                            OP.add, OP.min)
                    nc.vector.tensor_scalar(r0[:, sl_], slramp[:],
                                            phib[:, col:col + 1], 0.0,
                                            OP.add, OP.max)
                    nc.vector.tensor_scalar(r1[:, sl_], slramp[:],
                                            phim1[:, col:col + 1], 0.0,
                                            OP.add, OP.max)

                def c3(ap):
                    return ap.rearrange("p (b x) -> p b x", x=BLK)

                nc.vector.tensor_tensor(c3(mn[:]), c3(mn[:]),
                                        wsl(t1w, 1), OP.mult)
                nc.vector.tensor_tensor(c3(r0[:]), c3(r0[:]),
                                        wsl(t1w, 2), OP.mult)
                nc.vector.tensor_tensor(c3(r1[:]), c3(r1[:]),
                                        wsl(d2w, 2), OP.mult)

                # Rc = win2 + mn + r0 + r1 summed on the PE into PSUM;
                # exp reads PSUM; one wide Square per chunk.
                cnt = cb.tile([TP, CHW], BF16, tag="cnt")
                wtv = wt[:].rearrange("p (b l) -> p b l", l=WPIT)
                for i in range(4):
                    cps_ = psC.tile([TP, 512], F32, tag="rc")
                    sl = slice(512 * i, 512 * (i + 1))
                    nc.tensor.matmul(cps_[:],
                                     identB[:],
                                     wtv[:, 2 * i:2 * i + 2, 2:2 + BLK],
                                     start=True, stop=False)
                    nc.tensor.matmul(cps_[:], identB[:], mn[:, sl],
                                     start=False, stop=False)
                    nc.tensor.matmul(cps_[:], identB[:], r0[:, sl],
                                     start=False, stop=False)
                    nc.tensor.matmul(cps_[:], identB[:], r1[:, sl],
                                     start=False, stop=True)
                    nc.scalar.activation(cnt[:, sl], cps_[:],
                                         AF.Exp, scale=-1.0,
                                         bias=b1e[:, 0:1])
                nc.scalar.activation(
                    vqs[t][:, CHW * c:CHW * (c + 1)],
                    cnt[:], AF.Square, bias=qb[:, 0:1],
                    accum_out=msums[t][:, c:c + 1])

            wts = {(t, 0): gather_chunk(t, 0) for t in range(NT)}
            for c in range(NCH):
                for t in range(NT):
                    if c + 1 < NCH:
                        wts[t, c + 1] = gather_chunk(t, c + 1)
                    combine(t, c, wts.pop((t, c)))

            # stage C: per-tile epilogue; last fin quarter runs on DVE so the
            # tail drains on two engines.
            for t in range(NT):
                rs = t * TP
                d = P[t]
                Ct, Atc, qb, aq = d["Ct"], d["Atc"], d["qb"], d["aq"]
                vq, msum = vqs[t], msums[t]
                mtot = sp.tile([TP, 1], F32, tag="mtot")
                nc.vector.tensor_reduce(mtot[:], msum[:], mybir.AxisListType.X,
                                        OP.add)
                # Svq = Atc*(Ssq/W - q^2); meannl*omst = Svq_mean + Ct
                z1 = sp.tile([TP, 1], F32, tag="z1")
                nc.vector.tensor_scalar(z1[:], mtot[:], 1.0 / W, None, OP.mult)
                qq = sp.tile([TP, 1], F32, tag="qq")
                nc.vector.tensor_tensor(qq[:], qb[:], qb[:], OP.mult)
                nc.vector.tensor_tensor(z1[:], z1[:], qq[:], OP.subtract)
                nc.vector.tensor_tensor(z1[:], z1[:], Atc[:], OP.mult)
                nc.vector.tensor_scalar(z1[:], z1[:], Ct[:, 0:1], None, OP.add)
                stm = sp.tile([TP, 1], F32, tag="stm")
                nc.vector.tensor_tensor(stm[:], z1[:], d["ratio"][:], OP.mult)
                bias3 = sp.tile([TP, 1], F32, tag="bias3")
                nc.vector.tensor_scalar(bias3[:], stm[:], Ct[:, 0:1], None,
                                        OP.add)
                nc.vector.tensor_tensor(bias3[:], bias3[:], aq[:], OP.subtract)
                for g4 in range(4):
                    fin = fp.tile([TP, 2048], BF16, tag="fin")
                    if g4 == 3:
                        nc.vector.tensor_scalar(
                            fin[:], vq[:, 2048 * g4:2048 * (g4 + 1)],
                            Atc[:, 0:1], bias3[:, 0:1], OP.mult, OP.add)
                    else:
                        nc.scalar.activation(
                            fin[:], vq[:, 2048 * g4:2048 * (g4 + 1)],
                            AF.Identity, scale=Atc[:, 0:1],
                            bias=bias3[:, 0:1])
                    nc.sync.dma_start(out_e[rs:rs + TP,
                                            2048 * g4:2048 * (g4 + 1)],
                                      fin[:])

    nc.compile()
    return nc


def _prep(inputs):
    gas = np.asarray(inputs["gas_columns"], np.float32)
    ids = np.asarray(inputs["instrument_ids"]).astype(np.int64)
    nuis = np.asarray(inputs["nuisance_latent"], np.float32)
    am = np.asarray(inputs["air_mass"], np.float32)
    wl = np.asarray(inputs["wavelengths_nm"], np.float32)
    ab = np.asarray(inputs["absorption"], np.float32)
    cbm = np.asarray(inputs["continuum_basis"], np.float32)
    ray = np.asarray(inputs["rayleigh_od"], np.float32)
    emb = np.asarray(inputs["embed_table"], np.float32)
    w1 = np.asarray(inputs["w1"], np.float32)
    b1 = np.asarray(inputs["b1"], np.float32)
    w2 = np.asarray(inputs["w2"], np.float32)
    b2 = np.asarray(inputs["b2"], np.float32)

    wl0 = float(wl[0])
    dlam = float((wl[-1] - wl[0]) / (W - 1))

    Q = _basis()                                        # [15, M]
    Mmat = np.concatenate([ab, cbm[0:G], ray[None, :], cbm[G:G + 1]],
                          axis=0).astype(np.float32)    # [18, W]
    pad = K // 2
    Mp = np.pad(Mmat, ((0, 0), (pad, pad)), mode="edge")
    Mq = np.stack([
        sum(Mp[:, k:k + W] * Q[k, m] for k in range(K)) for m in range(M)
    ])                                                  # [M, 18, W]
    Mstack = Mq.reshape(M * 18, W).astype(np.float32)
    negMbar = (-Mstack.mean(axis=1, keepdims=True)).astype(ml_dtypes.bfloat16)
    MstackP = np.pad(Mstack, ((0, 0), (PADL, PADR)), mode="edge")
    MstackP = np.ascontiguousarray(MstackP).astype(ml_dtypes.bfloat16)
    qsum = np.ascontiguousarray(Q.sum(axis=0)[:, None]).astype(np.float32)

    jj = np.arange(CROWS)
    selS = (jj[None, :] // 18 == np.arange(M)[:, None]).astype(np.float32)
    selG = (jj[None, :] % 18 == np.arange(18)[:, None]).astype(np.float32)

    w1a = np.ascontiguousarray(w1[G:G + E])
    w1b = np.ascontiguousarray(w1[0:G])
    b1r = b1[None, :].astype(np.float32)
    w2s = np.concatenate([w2, b2[None, :]], 0)

    in_maps = []
    for c in range(NCORES):
        r = slice(c * BS, (c + 1) * BS)
        ohm = (ids[r][None, :] == np.arange(NI)[:, None]).astype(np.float32)
        gT = gas[r].T
        uT = np.concatenate([gT * am[r][None, :], gT, am[r][None, :],
                             np.ones((1, BS), np.float32)], 0)
        in_maps.append({
            "uT": np.ascontiguousarray(uT),
            "nuisT": np.ascontiguousarray(nuis[r].T),
            "onehotT": ohm,
            "embT16": emb,
            "w1a": w1a,
            "w1b": w1b,
            "b1r": b1r,
            "w2s": w2s,
            "MstackP": MstackP,
            "negMbar": negMbar,
            "Qs": Q.astype(np.float32),
            "qsum": qsum,
            "selS": selS,
            "selG": selG,
        })
    return in_maps, wl0, dlam


def kernel(**inputs):
    in_maps, wl0, dlam = _prep(inputs)
    key = (round(wl0, 6), round(dlam, 9))
    if key not in _CACHE:
        _CACHE[key] = _build(wl0, dlam)
    nc = _CACHE[key]
    res = run_bass_kernel_spmd(nc, in_maps, core_ids=list(range(NCORES)))
    outs = [np.asarray(res.results[i]["out"]).astype(np.float32)
            for i in range(NCORES)]
    return np.concatenate(outs, axis=0)


if __name__ == "__main__":
    rng = np.random.default_rng(0)
    ins = {
        "gas_columns": rng.random((B, G), dtype=np.float32),
        "instrument_ids": rng.integers(0, NI, B),
        "nuisance_latent": rng.standard_normal((B, G)).astype(np.float32),
        "air_mass": 1 + 2 * rng.random(B, dtype=np.float32),
        "wavelengths_nm": np.linspace(300, 400, W, dtype=np.float32),
        "absorption": 0.1 * rng.random((G, W), dtype=np.float32),
        "continuum_basis": 0.05 * rng.standard_normal((G + 1, W)).astype(np.float32),
        "rayleigh_od": rng.random(W, dtype=np.float32),
        "embed_table": rng.standard_normal((NI, E)).astype(np.float32),
        "w1": rng.standard_normal((G + E, 64)).astype(np.float32) / 12,
        "b1": np.zeros(64, np.float32),
        "w2": rng.standard_normal((64, 7)).astype(np.float32) / 8,
        "b2": np.zeros(7, np.float32),
    }
    out = kernel(**ins)
    print("out", out.shape, out.dtype, np.abs(out).mean())

